# revision 3
# baseline (speedup 1.0000x reference)
"""BiModalAttention Trainium2 kernel.

Full-input contract: kernel(mode1, mode2) -> [S, B, 2D] float32.
mode1/mode2: [S=1024, B=32, D=1024] float32.

Reference computation per batch b (m1 = mode1[:, b, :], m2 = mode2[:, b, :]):
    C1 = m1 @ m2.T                  # [S, S]
    a1 = softmax_rows(C1) @ m2 * m1
    a2 = softmax_rows(C1.T) @ m1 * m2
    out[:, b, :] = concat([a1, a2], -1)

Sharding: batch dim across 8 NeuronCores (4 batch elements per core).

Per-core kernel structure (per batch element):
  A. C1 = m1T.T @ m2T in fp32r (d-major operand layout, loaded via casting
     DMAs), PSUM->SBUF evacuation on ScalarE, row-max (negated) on VectorE.
  B. Broadcast row-maxes across partitions: DVE free-dim broadcast of the
     [P,1] column + PE transpose -> [*, S] tiles RM1B/RM2B.
  C. C2 = C1.T via PE transpose; evacuation fused with "+(-rm1[s])" on DVE;
     pre-subtraction row-max partials give rm2; ACT exp -> E1T (fp32r).
  D. E2T = exp(C1 + (-rm2[t] broadcast)) via DVE add + ACT exp (fp32r).
  E. AV matmuls per 256-wide d-chunk: o1 = E1T.T @ m2chunk, o2 = E2T.T @
     m1chunk (fp32r). Chunk 0 carries a ones column producing the softmax
     denominators Z in the same PSUM accumulation (two ones columns:
     the fp32r moving operand must have an even element count). Final evacuation is one
     DVE scalar_tensor_tensor: out = (psum * (1/Z)[part]) * gate_chunk.
"""

import numpy as np

import concourse.bacc as bacc
import concourse.mybir as mybir
import concourse.tile as tile
from concourse.masks import make_identity
from concourse.bass_utils import run_bass_kernel_spmd

S = 1024
D = 1024
B = 32
N_CORES = 8
BPC = B // N_CORES          # batch elements per core
P = 128                     # partitions
NK = S // P                 # k/t tiles per matrix (8)
NI = S // P                 # s tiles (8)
CW = 256                    # AV d-chunk width
NCH = D // CW               # AV chunks (4)

f32 = mybir.dt.float32
f32r = mybir.dt.float32r
AX = mybir.AxisListType
ALU = mybir.AluOpType
ACTF = mybir.ActivationFunctionType


def _emit_batch(nc, sb, ps, ident, j, m1t, m2t, m1n, m2n, outp):
    # ---- Phase A: scores C1[s, t] = sum_d m1[s,d] m2[t,d] ----
    m1t_sb = sb.tile([P, NK, S], f32r, tag="m1t", bufs=1, name=f"m1t_sb{j}")
    m2t_sb = sb.tile([P, NK, S], f32r, tag="m2t", bufs=1, name=f"m2t_sb{j}")
    nc.gpsimd.dma_start(out=m1t_sb, in_=m1t[j].rearrange("(k p) s -> p k s", p=P))
    nc.gpsimd.dma_start(out=m2t_sb, in_=m2t[j].rearrange("(k p) s -> p k s", p=P))

    c1 = []
    rm1 = sb.tile([P, NI], f32, tag="rm1", bufs=2, name=f"rm1_{j}")
    for i in range(NI):
        c1_i = sb.tile([P, S], f32, tag="c1", bufs=NI, name=f"c1_{j}_{i}")
        c1.append(c1_i)
        for n in range(2):
            pc = ps.tile([P, 512], f32, tag="c", bufs=3, name=f"pc{j}_{i}_{n}")
            for k in range(NK):
                nc.tensor.matmul(
                    pc,
                    m1t_sb[:, k, i * P:(i + 1) * P],
                    m2t_sb[:, k, n * 512:(n + 1) * 512],
                    start=(k == 0),
                    stop=(k == NK - 1),
                )
            nc.scalar.copy(out=c1_i[:, n * 512:(n + 1) * 512], in_=pc)
        # negated row max: rm1[:, i] = -max_t C1[s, t]
        nc.vector.tensor_reduce(rm1[:, i:i + 1], c1_i, axis=AX.X,
                                op=ALU.max, negate=True)

    # ---- Phase B: RM1B[t, s] = -rm1[s] (partition broadcast) ----
    def _bcast_rows(rm_cols, tag, nm):
        rmb = sb.tile([P, S], f32, tag=tag, bufs=1, name=nm)
        for g in range(2):
            pt = ps.tile([P, 512], f32, tag="t", bufs=2, name=f"{nm}_pt{g}")
            for q in range(4):
                i = g * 4 + q
                xb = sb.tile([P, P], f32, tag="xb", bufs=2, name=f"{nm}_xb{i}")
                nc.vector.tensor_copy(xb, rm_cols[:, i:i + 1].broadcast_to([P, P]))
                nc.tensor.transpose(pt[:, q * P:(q + 1) * P], xb, ident)
            nc.scalar.copy(out=rmb[:, g * 512:(g + 1) * 512], in_=pt)
        return rmb

    rm1b = _bcast_rows(rm1, "rm1b", f"rm1b_{j}")

    # ---- Phase C: C2 strips via PE transpose -> E1T = exp(C2 - rm1[s]) ----
    e1 = []
    rm2p = sb.tile([P, 2 * NK], f32, tag="rm2p", bufs=2, name=f"rm2p_{j}")
    rm2 = sb.tile([P, NK], f32, tag="rm2", bufs=2, name=f"rm2_{j}")
    for t in range(NK):
        e1_t = sb.tile([P, S], f32r, tag="e1", bufs=NK, name=f"e1_{j}_{t}")
        e1.append(e1_t)
        epre = sb.tile([P, S], f32, tag="epre", bufs=2, name=f"epre1_{j}_{t}")
        for g in range(2):
            pt = ps.tile([P, 512], f32, tag="t", bufs=2, name=f"ptr{j}_{t}_{g}")
            for q in range(4):
                i = g * 4 + q
                nc.tensor.transpose(pt[:, q * P:(q + 1) * P],
                                    c1[i][:, t * P:(t + 1) * P], ident)
            # negated row-max partial of raw C2 (pre-shift)
            nc.vector.tensor_reduce(rm2p[:, 2 * t + g:2 * t + g + 1], pt,
                                    axis=AX.X, op=ALU.max, negate=True)
            # evacuation fused with the shift: epre = C2 + (-rm1[s])
            nc.vector.tensor_add(epre[:, g * 512:(g + 1) * 512], pt,
                                 rm1b[:, g * 512:(g + 1) * 512])
        nc.vector.tensor_tensor(rm2[:, t:t + 1], rm2p[:, 2 * t:2 * t + 1],
                                rm2p[:, 2 * t + 1:2 * t + 2], op=ALU.min)
        nc.scalar.activation(e1_t, epre, ACTF.Exp)

    rm2b = _bcast_rows(rm2, "rm2b", f"rm2b_{j}")

    # ---- Phase D: E2T = exp(C1 - rm2[t]) ----
    e2 = []
    for i in range(NI):
        e2_i = sb.tile([P, S], f32r, tag="e2", bufs=NI, name=f"e2_{j}_{i}")
        e2.append(e2_i)
        epre2 = sb.tile([P, S], f32, tag="epre", bufs=2, name=f"epre2_{j}_{i}")
        nc.vector.tensor_add(epre2, c1[i], rm2b)
        nc.scalar.activation(e2_i, epre2, ACTF.Exp)

    # ---- Phase E: AV matmuls + gating, chunked along d ----
    invz1 = sb.tile([P, NI], f32, tag="invz1", bufs=2, name=f"invz1_{j}")
    invz2 = sb.tile([P, NI], f32, tag="invz2", bufs=2, name=f"invz2_{j}")
    for c in range(NCH):
        c0 = c * CW
        off = 2 if c == 0 else 0
        r2 = sb.tile([P, NK, CW + 2], f32r, tag="rhs", bufs=3, name=f"r2_{j}_{c}")
        r1 = sb.tile([P, NK, CW + 2], f32r, tag="rhs", bufs=3, name=f"r1_{j}_{c}")
        if c == 0:
            nc.vector.memset(r2[:, :, 0:2].bitcast(f32), 1.0)
            nc.vector.memset(r1[:, :, 0:2].bitcast(f32), 1.0)
        nc.gpsimd.dma_start(
            out=r2[:, :, off:off + CW],
            in_=m2n[j].rearrange("(k p) d -> p k d", p=P)[:, :, c0:c0 + CW])
        nc.gpsimd.dma_start(
            out=r1[:, :, off:off + CW],
            in_=m1n[j].rearrange("(k p) d -> p k d", p=P)[:, :, c0:c0 + CW])

        for i in range(NI):
            for (es, rhs, gate, invz, dbase) in (
                (e1, r2, r1, invz1, 0),
                (e2, r1, r2, invz2, D),
            ):
                pav = ps.tile([P, CW + off], f32, tag="av", bufs=3,
                              name=f"pav{j}_{c}_{i}_{dbase}")
                for k in range(NK):
                    nc.tensor.matmul(
                        pav,
                        es[k][:, i * P:(i + 1) * P],
                        rhs[:, k, 0:CW + off],
                        start=(k == 0),
                        stop=(k == NK - 1),
                    )
                if c == 0:
                    nc.vector.reciprocal(invz[:, i:i + 1], pav[:, 0:1])
                a_sb = sb.tile([P, CW], f32, tag="ao", bufs=4,
                               name=f"a{j}_{c}_{i}_{dbase}")
                nc.vector.scalar_tensor_tensor(
                    a_sb, pav[:, off:off + CW], invz[:, i:i + 1],
                    gate[:, i, off:off + CW].bitcast(f32),
                    op0=ALU.mult, op1=ALU.mult)
                nc.sync.dma_start(
                    out=outp[j, i * P:(i + 1) * P, dbase + c0:dbase + c0 + CW],
                    in_=a_sb)


def _build():
    nc = bacc.Bacc("TRN2", target_bir_lowering=False, debug=False,
                   num_devices=N_CORES)
    m1n = nc.dram_tensor("m1n", [BPC, S, D], f32, kind="ExternalInput").ap()
    m2n = nc.dram_tensor("m2n", [BPC, S, D], f32, kind="ExternalInput").ap()
    m1t = nc.dram_tensor("m1t", [BPC, D, S], f32, kind="ExternalInput").ap()
    m2t = nc.dram_tensor("m2t", [BPC, D, S], f32, kind="ExternalInput").ap()
    outp = nc.dram_tensor("out", [BPC, S, 2 * D], f32, kind="ExternalOutput").ap()

    with tile.TileContext(nc) as tc:
        with tc.tile_pool(name="consts", bufs=1) as consts, \
             tc.tile_pool(name="sb", bufs=1) as sb, \
             tc.tile_pool(name="ps", bufs=1, space="PSUM") as ps:
            ident = consts.tile([P, P], f32)
            make_identity(nc, ident)
            for j in range(BPC):
                _emit_batch(nc, sb, ps, ident, j, m1t, m2t, m1n, m2n, outp)
    nc.compile()
    return nc


_NC_CACHE = None


def _get_nc():
    global _NC_CACHE
    if _NC_CACHE is None:
        _NC_CACHE = _build()
    return _NC_CACHE


def kernel(mode1: np.ndarray, mode2: np.ndarray, _trace: bool = False,
           _result_box: dict | None = None) -> np.ndarray:
    mode1 = np.asarray(mode1, dtype=np.float32)
    mode2 = np.asarray(mode2, dtype=np.float32)

    m1n_all = np.ascontiguousarray(mode1.transpose(1, 0, 2))  # [B, S, D]
    m2n_all = np.ascontiguousarray(mode2.transpose(1, 0, 2))
    m1t_all = np.ascontiguousarray(mode1.transpose(1, 2, 0))  # [B, D, S]
    m2t_all = np.ascontiguousarray(mode2.transpose(1, 2, 0))

    nc = _get_nc()
    in_maps = []
    for c in range(N_CORES):
        lo, hi = c * BPC, (c + 1) * BPC
        in_maps.append({
            "m1n": m1n_all[lo:hi],
            "m2n": m2n_all[lo:hi],
            "m1t": m1t_all[lo:hi],
            "m2t": m2t_all[lo:hi],
        })

    r = run_bass_kernel_spmd(nc, in_maps, list(range(N_CORES)), trace=_trace)
    if _result_box is not None:
        _result_box["result"] = r

    out = np.empty((S, B, 2 * D), dtype=np.float32)
    for c in range(N_CORES):
        res = r.results[c]["out"]  # [BPC, S, 2D]
        out[:, c * BPC:(c + 1) * BPC, :] = res.transpose(1, 0, 2)
    return out
